# revision 26
# baseline (speedup 1.0000x reference)
# Multi-head attention (BS=2, SL=2048, D=1024, NH=16) on 8 NeuronCores.
#
# Sharding: batch (2) x query-range (4): core c owns batch c//4 and query rows
# [512*(c%4), 512*(c%4+1)). Each core computes the k/v projections for the
# full sequence of its batch (replicated within the batch group -- measured
# collectives for sharing them cost 50-85us each plus a ~55us first-call
# penalty on this fabric, so recomputing beats communicating; fp8 projections
# were also measured at 4.7e-2 end-to-end error vs the 2e-2 gate, so all
# matmuls stay bf16), the q projection for its own rows, all 16 heads of
# attention for its rows, and a complete 512-row slice of the output
# projection. The output is a pure concatenation of the 8 per-core slices.
#
# Schedule: the kernel is PE-bound (~240us of matmul streams at ~216ns per
# 512-col stream), so everything is software-pipelined around the scores
# stream that feeds ScalarE's exp (the only other >100us engine):
#   prolog:  qproj, kproj pair 0
#   pair  p: scores(p) chunks interleaved per-chunk with ctx(p-1),
#            kproj(p+1) (4 psum groups spread across the 16 chunk slots),
#            and vproj (16 chunks split across pairs 0-1)
#   tail:    ctx(7), normalize(7), out projection in 2 waves
# kT tiles live in a 3-deep ring (scores(p) is kT[p]'s only consumer);
# pt (exp'd scores) pool depth covers the 1-pair scores->ctx lag.
#
# Layout tricks:
#  - q/k kept transposed ([feat, seq], head-dim on partitions) so scores^T
#    comes out with k-position on partitions and the softmax reduction rides
#    the ctx matmul (ones column in v) instead of a cross-partition reduce.
#  - HD=64 means scores matmuls only use half the PE contraction rows, so
#    head PAIRS run concurrently in 64x128 row-tiled mode (tile_position).
#  - exp runs on ScalarE straight out of PSUM in [128,1024] tiles (two heads
#    merged per instruction to amortize the ~260ns ACT bubble).
#
# Self-contained: hardcodes shapes; host preps shards (transpose/cast/slice).

import functools

import numpy as np
import ml_dtypes

import concourse.bass as bass
import concourse.mybir as mybir
import concourse.tile as tile
from concourse import bacc
from concourse.bass_utils import run_bass_kernel_spmd

BS, SL, D, NH, HD = 2, 2048, 1024, 16, 64
SCALE = D ** -0.5  # reference scales q by full model dim
NCORES = 8
GROUP = 4                 # cores per batch
QB = SL // GROUP          # q-rows per core = 512
NPAIR = NH // 2           # head pairs = 8
VW = NH * (HD + 1)        # v_ext width = 1040

BF16 = mybir.dt.bfloat16
F32 = mybir.dt.float32
NKD = D // 128            # contraction chunks over D = 8
NSEQ = SL // 128          # seq chunks = 16


def _attention_body(nc, tc):
    xT = nc.dram_tensor("xT", [D, SL], BF16, kind="ExternalInput")
    xTq = nc.dram_tensor("xTq", [D, QB], BF16, kind="ExternalInput")
    wq = nc.dram_tensor("wq", [D, D], BF16, kind="ExternalInput")
    wk = nc.dram_tensor("wk", [D, D], BF16, kind="ExternalInput")
    wv = nc.dram_tensor("wv", [D, D], BF16, kind="ExternalInput")
    wo = nc.dram_tensor("wo", [D, D], BF16, kind="ExternalInput")
    out = nc.dram_tensor("out", [QB, D], F32, kind="ExternalOutput")

    Exp = mybir.ActivationFunctionType.Exp

    with (
        tc.tile_pool(name="x", bufs=1) as xpool,
        tc.tile_pool(name="w", bufs=1) as wpool,
        tc.tile_pool(name="qt", bufs=1) as qtpool,
        tc.tile_pool(name="kt", bufs=3) as ktpool,
        tc.tile_pool(name="vx", bufs=1) as vpool,
        tc.tile_pool(name="ctx", bufs=1) as ctxpool,
        tc.tile_pool(name="sm", bufs=3) as smpool,
        tc.tile_pool(name="oproj", bufs=2) as opool,
        # PSUM budget (8 banks): mm 2x[128,512] + scores 2x[128,1024] + ctx 2x[65,512]
        tc.tile_pool(name="ps_mm", bufs=2, space="PSUM") as ps_mm,
        tc.tile_pool(name="ps_s", bufs=2, space="PSUM") as ps_s,
        tc.tile_pool(name="ps_c", bufs=2, space="PSUM") as ps_c,
    ):
        # input loads: qproj operands first, then kproj's, then wv, wo.
        pw_scope = tc.tile_pool(name="pw", bufs=1)
        pw = pw_scope.__enter__()
        xq_sb, wq_sb, xT_sb, wk_sb, wv_sb, wo_sb = [], [], [], [], [], []
        for i in range(NKD):
            t = pw.tile([128, QB], BF16, tag=f"xq{i}", name=f"xq{i}")
            nc.sync.dma_start(t[:], xTq[i * 128:(i + 1) * 128, :])
            xq_sb.append(t)
            t = pw.tile([128, D], BF16, tag=f"wq{i}", name=f"wq{i}")
            nc.sync.dma_start(t[:], wq[i * 128:(i + 1) * 128, :])
            wq_sb.append(t)
        for i in range(NKD):
            t = xpool.tile([128, SL], BF16, tag=f"xT{i}", name=f"xT{i}")
            nc.sync.dma_start(t[:], xT[i * 128:(i + 1) * 128, :])
            xT_sb.append(t)
            t = wpool.tile([128, D], BF16, tag=f"wk{i}", name=f"wk{i}")
            nc.sync.dma_start(t[:], wk[i * 128:(i + 1) * 128, :])
            wk_sb.append(t)
        for i in range(NKD):
            t = wpool.tile([128, D], BF16, tag=f"wv{i}", name=f"wv{i}")
            nc.sync.dma_start(t[:], wv[i * 128:(i + 1) * 128, :])
            wv_sb.append(t)
        for p in range(NPAIR):
            t = wpool.tile([128, D], BF16, tag=f"wo{p}", name=f"wo{p}")
            nc.sync.dma_start(t[:], wo[p * 128:(p + 1) * 128, :])
            wo_sb.append(t)

        # qT [D qfeat, QB] (wq pre-scaled on host)
        qT_sb = []
        for p in range(NPAIR):
            t = qtpool.tile([128, QB], BF16, tag=f"qT{p}", name=f"qT{p}")
            ps = ps_mm.tile([128, 512], F32, tag="mm512")
            for kk in range(NKD):
                nc.tensor.matmul(
                    ps[:],
                    lhsT=wq_sb[kk][:, p * 128:(p + 1) * 128],
                    rhs=xq_sb[kk][:],
                    start=(kk == 0),
                    stop=(kk == NKD - 1),
                )
            nc.vector.tensor_copy(t[:], ps[:])
            qT_sb.append(t)
        pw_scope.__exit__(None, None, None)
        # pt pool reuses the qproj operands' SBUF
        pt_scope = tc.tile_pool(name="pt", bufs=NSEQ + 6)
        ptpool = pt_scope.__enter__()

        kT_sb = [None] * NPAIR

        def emit_kproj_group(p, n):
            # one [128, 512] seq-block of pair p's kT
            if kT_sb[p] is None:
                kT_sb[p] = ktpool.tile([128, SL], BF16, tag="kT", name=f"kT{p}")
            ps = ps_mm.tile([128, 512], F32, tag="mm512")
            for kk in range(NKD):
                nc.tensor.matmul(
                    ps[:],
                    lhsT=wk_sb[kk][:, p * 128:(p + 1) * 128],
                    rhs=xT_sb[kk][:, n * 512:(n + 1) * 512],
                    start=(kk == 0),
                    stop=(kk == NKD - 1),
                )
            nc.vector.tensor_copy(kT_sb[p][:, n * 512:(n + 1) * 512], ps[:])

        v_sb = [None] * NSEQ

        def emit_vproj_chunk(m):
            t = vpool.tile([128, VW], BF16, tag=f"v{m}", name=f"v{m}")
            v_sb[m] = t
            for n2 in range(2):
                ps = ps_mm.tile([128, 512], F32, tag="mm512")
                for kk in range(NKD):
                    nc.tensor.matmul(
                        ps[:],
                        lhsT=xT_sb[kk][:, m * 128:(m + 1) * 128],
                        rhs=wv_sb[kk][:, n2 * 512:(n2 + 1) * 512],
                        start=(kk == 0),
                        stop=(kk == NKD - 1),
                    )
                dst = t[:].rearrange("p (h c) -> p h c", c=HD + 1)[
                    :, n2 * 8:(n2 + 1) * 8, 1:1 + HD
                ]
                src = ps[:].rearrange("p (h c) -> p h c", c=HD)
                nc.vector.tensor_copy(dst, src)
            nc.vector.memset(
                t[:].rearrange("p (h c) -> p h c", c=HD + 1)[:, :, 0:1], 1.0
            )

        # ---- attention ----
        ctxT_sb = []
        for p in range(NPAIR):
            ctxT_sb.append(
                ctxpool.tile([128, QB], BF16, tag=f"ctxT{p}", name=f"ctxT{p}")
            )

        def emit_normalize(p, cps):
            # normalize rows 1..64 by row 0 (ones-row dot = exp row-sum)
            for hh in range(2):
                recip = smpool.tile(
                    [1, 512], F32, tag="recip", name=f"recip{p}_{hh}"
                )
                nc.vector.reciprocal_approx_fast(recip[:], cps[hh][0:1, :])
                rbc = smpool.tile([65, 512], F32, tag="rbc", name=f"rbc{p}_{hh}")
                nc.gpsimd.partition_broadcast(rbc[:], recip[:])
                stage = smpool.tile(
                    [65, 512], BF16, tag="stage", name=f"stage{p}_{hh}"
                )
                nc.vector.tensor_tensor(
                    stage[0:64, :], cps[hh][0:64, :], rbc[0:64, :],
                    mybir.AluOpType.mult,
                )
                nc.vector.tensor_tensor(
                    stage[64:65, :], cps[hh][64:65, :], rbc[64:65, :],
                    mybir.AluOpType.mult,
                )
                nc.sync.dma_start(
                    ctxT_sb[p][hh * 64:(hh + 1) * 64, :], stage[1:65, :]
                )

        pts = {}
        cps_open = {}

        def emit_scores_chunk(p, m):
            ps = ps_s.tile([128, 1024], F32, tag="scores")
            nc.tensor.matmul(
                ps[:, 0:512],
                lhsT=kT_sb[p][0:64, m * 128:(m + 1) * 128],
                rhs=qT_sb[p][0:64, :],
                start=True, stop=True,
                tile_position=(0, 0),
            )
            nc.tensor.matmul(
                ps[:, 512:1024],
                lhsT=kT_sb[p][64:128, m * 128:(m + 1) * 128],
                rhs=qT_sb[p][64:128, :],
                start=True, stop=True,
                tile_position=(64, 0),
            )
            pt = ptpool.tile([128, 1024], BF16, tag="pt")
            nc.scalar.activation(pt[:], ps[:], Exp)
            pts[(p, m)] = pt

        def emit_ctx_chunk(p, m):
            if m == 0:
                cps_open[p] = [
                    ps_c.tile([65, 512], F32, tag="ctx", name=f"cps{p}_{hh}")
                    for hh in range(2)
                ]
            cps = cps_open[p]
            for hh in range(2):
                h = 2 * p + hh
                nc.tensor.matmul(
                    cps[hh][:],
                    lhsT=v_sb[m][:, h * 65:(h + 1) * 65],
                    rhs=pts[(p, m)][:, hh * 512:(hh + 1) * 512],
                    start=(m == 0),
                    stop=(m == NSEQ - 1),
                )
            del pts[(p, m)]
            if m == NSEQ - 1:
                emit_normalize(p, cps_open.pop(p))

        # prolog: kproj pair 0
        for n in range(SL // 512):
            emit_kproj_group(0, n)

        # main loop: per chunk slot m of pair p emit scores(p,m), one
        # kproj(p+1) group per 4 slots, vproj chunks (pairs 0-1 only, one
        # per even slot -> v[m'] is emitted before ctx(1, m') consumes it),
        # and ctx(p-1, m).
        # slot order: filler (kproj/vproj) and ctx BEFORE the scores matmul,
        # so the exp drain of the previous scores chunk has ~1us of PE work
        # to hide behind instead of stalling the scores psum WAR. ctx chunks
        # chase the scores stream with a 4-slot lag (queue) instead of a
        # full-pair lag, so only ~4 ctx chunks trail the final scores chunk.
        # vproj runs one chunk per slot of pair 0 so v[m] precedes ctx(0,m).
        ctxq = []

        def pump_ctx(lag):
            while len(ctxq) > lag:
                pp, mm = ctxq.pop(0)
                emit_ctx_chunk(pp, mm)

        for p in range(NPAIR):
            for m in range(NSEQ):
                if p < NPAIR - 1 and m % 4 == 3:
                    emit_kproj_group(p + 1, m // 4)
                if p == 0:
                    emit_vproj_chunk(m)
                pump_ctx(3)
                emit_scores_chunk(p, m)
                ctxq.append((p, m))
        pump_ctx(0)
        pt_scope.__exit__(None, None, None)

        # ---- output projection: out[QB, D] = ctx[QB, D] @ w_out ----
        # 8 psum groups in 2 waves of 4 (2 slots from ps_mm + 2 borrowed from
        # the now-idle scores pool). Each wave emits every group's pair-0..6
        # accumulation before any pair-7-dependent matmul, so the last pair's
        # normalize chain overlaps real PE work instead of stalling the FIFO.
        groups = [(n, mq) for n in range(D // 512) for mq in range(QB // 128)]
        for wave in range(2):
            wgroups = groups[wave * 4:(wave + 1) * 4]
            pss = []
            for gi, (n, mq) in enumerate(wgroups):
                if gi < 2:
                    ps = ps_mm.tile([128, 512], F32, tag="mm512")
                else:
                    ps = ps_s.tile([128, 512], F32, tag="scores")
                pss.append(ps)
                for p in range(NPAIR - 1):
                    nc.tensor.matmul(
                        ps[:],
                        lhsT=ctxT_sb[p][:, mq * 128:(mq + 1) * 128],
                        rhs=wo_sb[p][:, n * 512:(n + 1) * 512],
                        start=(p == 0),
                        stop=False,
                    )
            for gi, (n, mq) in enumerate(wgroups):
                ps = pss[gi]
                nc.tensor.matmul(
                    ps[:],
                    lhsT=ctxT_sb[NPAIR - 1][:, mq * 128:(mq + 1) * 128],
                    rhs=wo_sb[NPAIR - 1][:, n * 512:(n + 1) * 512],
                    start=False,
                    stop=True,
                )
                o = opool.tile([128, 512], F32, tag="osb")
                nc.vector.tensor_copy(o[:], ps[:])
                nc.sync.dma_start(
                    out[mq * 128:(mq + 1) * 128, n * 512:(n + 1) * 512], o[:]
                )


@functools.lru_cache(maxsize=2)
def _build(debug_taps=False):
    nc = bacc.Bacc(
        "TRN2",
        target_bir_lowering=False,
        debug=False,
        enable_asserts=True,
        num_devices=NCORES,
    )
    with tile.TileContext(nc) as tc:
        _attention_body(nc, tc)
    nc.compile()
    return nc


def make_in_maps(input_sequence, w_qkv, w_out):
    bf16 = ml_dtypes.bfloat16
    x = np.asarray(input_sequence, dtype=np.float32)
    w_qkv = np.asarray(w_qkv, dtype=np.float32)
    w_out = np.asarray(w_out, dtype=np.float32)

    xT = [np.ascontiguousarray(x[b].T).astype(bf16) for b in range(BS)]
    wq_ = np.ascontiguousarray(w_qkv[:, :D] * SCALE).astype(bf16)
    wk_ = np.ascontiguousarray(w_qkv[:, D:2 * D]).astype(bf16)
    wv_ = np.ascontiguousarray(w_qkv[:, 2 * D:]).astype(bf16)
    wo_ = np.ascontiguousarray(w_out).astype(bf16)
    in_maps = []
    for c in range(NCORES):
        b, r = divmod(c, GROUP)
        in_maps.append({
            "xT": xT[b],
            "xTq": np.ascontiguousarray(xT[b][:, r * QB:(r + 1) * QB]),
            "wq": wq_, "wk": wk_, "wv": wv_, "wo": wo_,
        })
    return in_maps


def assemble_output(results):
    out = np.empty((BS, SL, D), dtype=np.float32)
    for c in range(NCORES):
        b, r = divmod(c, GROUP)
        out[b, r * QB:(r + 1) * QB, :] = results[c]["out"]
    return out


def kernel(input_sequence, w_qkv, w_out, _trace=False, _results=[None]):
    nc = _build()
    in_maps = make_in_maps(input_sequence, w_qkv, w_out)
    res = run_bass_kernel_spmd(
        nc, in_maps, core_ids=list(range(NCORES)), trace=_trace
    )
    _results[0] = res
    return assemble_output(res.results)


# revision 27
# speedup vs baseline: 1.2013x; 1.2013x over previous
# Multi-head attention (BS=2, SL=2048, D=1024, NH=16) on 8 NeuronCores.
#
# Sharding: batch (2) x query-range (4): core c owns batch c//4 and query rows
# [512*(c%4), 512*(c%4+1)). Each core computes the k/v projections for the
# full sequence of its batch (replicated within the batch group -- measured
# collectives for sharing them cost 50-85us each plus a ~55us first-call
# penalty on this fabric, so recomputing beats communicating; fp8 projections
# were also measured at 4.7e-2 end-to-end error vs the 2e-2 gate, so all
# matmuls stay bf16), the q projection for its own rows, all 16 heads of
# attention for its rows, and a complete 512-row slice of the output
# projection. The output is a pure concatenation of the 8 per-core slices.
#
# Schedule: the kernel is PE-bound (~240us of matmul streams at ~216ns per
# 512-col stream), so everything is software-pipelined around the scores
# stream that feeds ScalarE's exp (the only other >100us engine):
#   prolog:  qproj, kproj pair 0
#   pair  p: scores(p) chunks interleaved per-chunk with ctx(p-1),
#            kproj(p+1) (4 psum groups spread across the 16 chunk slots),
#            and vproj (16 chunks split across pairs 0-1)
#   tail:    ctx(7), normalize(7), out projection in 2 waves
# kT tiles live in a 3-deep ring (scores(p) is kT[p]'s only consumer);
# pt (exp'd scores) pool depth covers the 1-pair scores->ctx lag.
#
# Layout tricks:
#  - q/k kept transposed ([feat, seq], head-dim on partitions) so scores^T
#    comes out with k-position on partitions and the softmax reduction rides
#    the ctx matmul (ones column in v) instead of a cross-partition reduce.
#  - HD=64 means scores matmuls only use half the PE contraction rows, so
#    head PAIRS run concurrently in 64x128 row-tiled mode (tile_position).
#  - exp runs on ScalarE straight out of PSUM in [128,1024] tiles (two heads
#    merged per instruction to amortize the ~260ns ACT bubble).
#
# Self-contained: hardcodes shapes; host preps shards (transpose/cast/slice).

import functools

import numpy as np
import ml_dtypes

import concourse.bass as bass
import concourse.mybir as mybir
import concourse.tile as tile
from concourse import bacc
from concourse.bass_utils import run_bass_kernel_spmd

BS, SL, D, NH, HD = 2, 2048, 1024, 16, 64
SCALE = D ** -0.5  # reference scales q by full model dim
NCORES = 8
GROUP = 4                 # cores per batch
QB = SL // GROUP          # q-rows per core = 512
NPAIR = NH // 2           # head pairs = 8
VW = NH * (HD + 1)        # v_ext width = 1040

BF16 = mybir.dt.bfloat16
F32 = mybir.dt.float32
NKD = D // 128            # contraction chunks over D = 8
NSEQ = SL // 128          # seq chunks = 16


def _attention_body(nc, tc):
    xT = nc.dram_tensor("xT", [D, SL], BF16, kind="ExternalInput")
    xTq = nc.dram_tensor("xTq", [D, QB], BF16, kind="ExternalInput")
    wq = nc.dram_tensor("wq", [D, D], BF16, kind="ExternalInput")
    wk = nc.dram_tensor("wk", [D, D], BF16, kind="ExternalInput")
    wv = nc.dram_tensor("wv", [D, D], BF16, kind="ExternalInput")
    wo = nc.dram_tensor("wo", [D, D], BF16, kind="ExternalInput")
    out = nc.dram_tensor("out", [QB, D], F32, kind="ExternalOutput")

    Exp = mybir.ActivationFunctionType.Exp

    with (
        tc.tile_pool(name="x", bufs=1) as xpool,
        tc.tile_pool(name="w", bufs=1) as wpool,
        tc.tile_pool(name="qt", bufs=1) as qtpool,
        tc.tile_pool(name="kt", bufs=3) as ktpool,
        tc.tile_pool(name="vx", bufs=1) as vpool,
        tc.tile_pool(name="ctx", bufs=1) as ctxpool,
        tc.tile_pool(name="sm", bufs=3) as smpool,
        tc.tile_pool(name="oproj", bufs=2) as opool,
        # PSUM budget (8 banks): mm 2x[128,512] + scores 2x[128,1024] + ctx 2x[65,512]
        tc.tile_pool(name="ps_mm", bufs=2, space="PSUM") as ps_mm,
        tc.tile_pool(name="ps_s", bufs=2, space="PSUM") as ps_s,
        tc.tile_pool(name="ps_c", bufs=2, space="PSUM") as ps_c,
    ):
        # input loads: qproj operands first, then kproj's, then wv, wo.
        pw_scope = tc.tile_pool(name="pw", bufs=1)
        pw = pw_scope.__enter__()
        xq_sb, wq_sb, xT_sb, wk_sb, wv_sb, wo_sb = [], [], [], [], [], []
        for i in range(NKD):
            t = pw.tile([128, QB], BF16, tag=f"xq{i}", name=f"xq{i}")
            nc.sync.dma_start(t[:], xTq[i * 128:(i + 1) * 128, :])
            xq_sb.append(t)
            t = pw.tile([128, D], BF16, tag=f"wq{i}", name=f"wq{i}")
            nc.sync.dma_start(t[:], wq[i * 128:(i + 1) * 128, :])
            wq_sb.append(t)
        for i in range(NKD):
            t = xpool.tile([128, SL], BF16, tag=f"xT{i}", name=f"xT{i}")
            nc.sync.dma_start(t[:], xT[i * 128:(i + 1) * 128, :])
            xT_sb.append(t)
            t = wpool.tile([128, D], BF16, tag=f"wk{i}", name=f"wk{i}")
            nc.sync.dma_start(t[:], wk[i * 128:(i + 1) * 128, :])
            wk_sb.append(t)
        for i in range(NKD):
            t = wpool.tile([128, D], BF16, tag=f"wv{i}", name=f"wv{i}")
            nc.sync.dma_start(t[:], wv[i * 128:(i + 1) * 128, :])
            wv_sb.append(t)
        for p in range(NPAIR):
            t = wpool.tile([128, D], BF16, tag=f"wo{p}", name=f"wo{p}")
            nc.sync.dma_start(t[:], wo[p * 128:(p + 1) * 128, :])
            wo_sb.append(t)

        # qT [D qfeat, QB] (wq pre-scaled on host)
        qT_sb = []
        for p in range(NPAIR):
            t = qtpool.tile([128, QB], BF16, tag=f"qT{p}", name=f"qT{p}")
            ps = ps_mm.tile([128, 512], F32, tag="mm512")
            for kk in range(NKD):
                nc.tensor.matmul(
                    ps[:],
                    lhsT=wq_sb[kk][:, p * 128:(p + 1) * 128],
                    rhs=xq_sb[kk][:],
                    start=(kk == 0),
                    stop=(kk == NKD - 1),
                )
            nc.vector.tensor_copy(t[:], ps[:])
            qT_sb.append(t)
        pw_scope.__exit__(None, None, None)
        # pt pool reuses the qproj operands' SBUF
        pt_scope = tc.tile_pool(name="pt", bufs=NSEQ + 6)
        ptpool = pt_scope.__enter__()

        kT_sb = [None] * NPAIR

        def emit_kproj_group(p, n):
            # one [128, 512] seq-block of pair p's kT
            if kT_sb[p] is None:
                kT_sb[p] = ktpool.tile([128, SL], BF16, tag="kT", name=f"kT{p}")
            ps = ps_mm.tile([128, 512], F32, tag="mm512")
            for kk in range(NKD):
                nc.tensor.matmul(
                    ps[:],
                    lhsT=wk_sb[kk][:, p * 128:(p + 1) * 128],
                    rhs=xT_sb[kk][:, n * 512:(n + 1) * 512],
                    start=(kk == 0),
                    stop=(kk == NKD - 1),
                )
            nc.vector.tensor_copy(kT_sb[p][:, n * 512:(n + 1) * 512], ps[:])

        v_sb = [None] * NSEQ

        def emit_vproj_chunk(m):
            t = vpool.tile([128, VW], BF16, tag=f"v{m}", name=f"v{m}")
            v_sb[m] = t
            for n2 in range(2):
                ps = ps_mm.tile([128, 512], F32, tag="mm512")
                for kk in range(NKD):
                    nc.tensor.matmul(
                        ps[:],
                        lhsT=xT_sb[kk][:, m * 128:(m + 1) * 128],
                        rhs=wv_sb[kk][:, n2 * 512:(n2 + 1) * 512],
                        start=(kk == 0),
                        stop=(kk == NKD - 1),
                    )
                dst = t[:].rearrange("p (h c) -> p h c", c=HD + 1)[
                    :, n2 * 8:(n2 + 1) * 8, 1:1 + HD
                ]
                src = ps[:].rearrange("p (h c) -> p h c", c=HD)
                nc.vector.tensor_copy(dst, src)
            nc.vector.memset(
                t[:].rearrange("p (h c) -> p h c", c=HD + 1)[:, :, 0:1], 1.0
            )

        # ---- attention ----
        ctxT_sb = []
        for p in range(NPAIR):
            ctxT_sb.append(
                ctxpool.tile([128, QB], BF16, tag=f"ctxT{p}", name=f"ctxT{p}")
            )

        def emit_normalize(p, cps):
            # normalize rows 1..64 by row 0 (ones-row dot = exp row-sum)
            for hh in range(2):
                recip = smpool.tile(
                    [1, 512], F32, tag="recip", name=f"recip{p}_{hh}"
                )
                nc.vector.reciprocal_approx_fast(recip[:], cps[hh][0:1, :])
                rbc = smpool.tile([65, 512], F32, tag="rbc", name=f"rbc{p}_{hh}")
                nc.gpsimd.partition_broadcast(rbc[:], recip[:])
                stage = smpool.tile(
                    [65, 512], BF16, tag="stage", name=f"stage{p}_{hh}"
                )
                nc.vector.tensor_tensor(
                    stage[0:64, :], cps[hh][0:64, :], rbc[0:64, :],
                    mybir.AluOpType.mult,
                )
                nc.vector.tensor_tensor(
                    stage[64:65, :], cps[hh][64:65, :], rbc[64:65, :],
                    mybir.AluOpType.mult,
                )
                nc.sync.dma_start(
                    ctxT_sb[p][hh * 64:(hh + 1) * 64, :], stage[1:65, :]
                )

        pts = {}
        cps_open = {}

        def emit_scores_chunk(p, m):
            ps = ps_s.tile([128, 1024], F32, tag="scores")
            nc.tensor.matmul(
                ps[:, 0:512],
                lhsT=kT_sb[p][0:64, m * 128:(m + 1) * 128],
                rhs=qT_sb[p][0:64, :],
                start=True, stop=True,
                tile_position=(0, 0),
            )
            nc.tensor.matmul(
                ps[:, 512:1024],
                lhsT=kT_sb[p][64:128, m * 128:(m + 1) * 128],
                rhs=qT_sb[p][64:128, :],
                start=True, stop=True,
                tile_position=(64, 0),
            )
            pt = ptpool.tile([128, 1024], BF16, tag="pt")
            nc.scalar.activation(pt[:], ps[:], Exp)
            pts[(p, m)] = pt

        def emit_ctx_chunk(p, m):
            if m == 0:
                cps_open[p] = [
                    ps_c.tile([65, 512], F32, tag="ctx", name=f"cps{p}_{hh}")
                    for hh in range(2)
                ]
            cps = cps_open[p]
            for hh in range(2):
                h = 2 * p + hh
                nc.tensor.matmul(
                    cps[hh][:],
                    lhsT=v_sb[m][:, h * 65:(h + 1) * 65],
                    rhs=pts[(p, m)][:, hh * 512:(hh + 1) * 512],
                    start=(m == 0),
                    stop=(m == NSEQ - 1),
                )
            del pts[(p, m)]
            if m == NSEQ - 1:
                emit_normalize(p, cps_open.pop(p))

        # prolog: kproj pair 0
        for n in range(SL // 512):
            emit_kproj_group(0, n)

        # main loop: per chunk slot m of pair p emit scores(p,m), one
        # kproj(p+1) group per 4 slots, vproj chunks (pairs 0-1 only, one
        # per even slot -> v[m'] is emitted before ctx(1, m') consumes it),
        # and ctx(p-1, m).
        # slot order: filler (kproj/vproj) and ctx BEFORE the scores matmul,
        # so the exp drain of the previous scores chunk has ~1us of PE work
        # to hide behind instead of stalling the scores psum WAR.
        for p in range(NPAIR):
            for m in range(NSEQ):
                if p < NPAIR - 1 and m % 4 == 3:
                    emit_kproj_group(p + 1, m // 4)
                if p < 2 and m % 2 == 1:
                    emit_vproj_chunk(8 * p + m // 2)
                if p >= 1:
                    emit_ctx_chunk(p - 1, m)
                emit_scores_chunk(p, m)
        for m in range(NSEQ):
            emit_ctx_chunk(NPAIR - 1, m)
        pt_scope.__exit__(None, None, None)

        # ---- output projection: out[QB, D] = ctx[QB, D] @ w_out ----
        # 8 psum groups in 2 waves of 4 (2 slots from ps_mm + 2 borrowed from
        # the now-idle scores pool). Each wave emits every group's pair-0..6
        # accumulation before any pair-7-dependent matmul, so the last pair's
        # normalize chain overlaps real PE work instead of stalling the FIFO.
        groups = [(n, mq) for n in range(D // 512) for mq in range(QB // 128)]
        for wave in range(2):
            wgroups = groups[wave * 4:(wave + 1) * 4]
            pss = []
            for gi, (n, mq) in enumerate(wgroups):
                if gi < 2:
                    ps = ps_mm.tile([128, 512], F32, tag="mm512")
                else:
                    ps = ps_s.tile([128, 512], F32, tag="scores")
                pss.append(ps)
                for p in range(NPAIR - 1):
                    nc.tensor.matmul(
                        ps[:],
                        lhsT=ctxT_sb[p][:, mq * 128:(mq + 1) * 128],
                        rhs=wo_sb[p][:, n * 512:(n + 1) * 512],
                        start=(p == 0),
                        stop=False,
                    )
            for gi, (n, mq) in enumerate(wgroups):
                ps = pss[gi]
                nc.tensor.matmul(
                    ps[:],
                    lhsT=ctxT_sb[NPAIR - 1][:, mq * 128:(mq + 1) * 128],
                    rhs=wo_sb[NPAIR - 1][:, n * 512:(n + 1) * 512],
                    start=False,
                    stop=True,
                )
                o = opool.tile([128, 512], F32, tag="osb")
                nc.vector.tensor_copy(o[:], ps[:])
                nc.sync.dma_start(
                    out[mq * 128:(mq + 1) * 128, n * 512:(n + 1) * 512], o[:]
                )


@functools.lru_cache(maxsize=2)
def _build(debug_taps=False):
    nc = bacc.Bacc(
        "TRN2",
        target_bir_lowering=False,
        debug=False,
        enable_asserts=True,
        num_devices=NCORES,
    )
    with tile.TileContext(nc) as tc:
        _attention_body(nc, tc)
    nc.compile()
    return nc


def make_in_maps(input_sequence, w_qkv, w_out):
    bf16 = ml_dtypes.bfloat16
    x = np.asarray(input_sequence, dtype=np.float32)
    w_qkv = np.asarray(w_qkv, dtype=np.float32)
    w_out = np.asarray(w_out, dtype=np.float32)

    xT = [np.ascontiguousarray(x[b].T).astype(bf16) for b in range(BS)]
    wq_ = np.ascontiguousarray(w_qkv[:, :D] * SCALE).astype(bf16)
    wk_ = np.ascontiguousarray(w_qkv[:, D:2 * D]).astype(bf16)
    wv_ = np.ascontiguousarray(w_qkv[:, 2 * D:]).astype(bf16)
    wo_ = np.ascontiguousarray(w_out).astype(bf16)
    in_maps = []
    for c in range(NCORES):
        b, r = divmod(c, GROUP)
        in_maps.append({
            "xT": xT[b],
            "xTq": np.ascontiguousarray(xT[b][:, r * QB:(r + 1) * QB]),
            "wq": wq_, "wk": wk_, "wv": wv_, "wo": wo_,
        })
    return in_maps


def assemble_output(results):
    out = np.empty((BS, SL, D), dtype=np.float32)
    for c in range(NCORES):
        b, r = divmod(c, GROUP)
        out[b, r * QB:(r + 1) * QB, :] = results[c]["out"]
    return out


def kernel(input_sequence, w_qkv, w_out, _trace=False, _results=[None]):
    nc = _build()
    in_maps = make_in_maps(input_sequence, w_qkv, w_out)
    res = run_bass_kernel_spmd(
        nc, in_maps, core_ids=list(range(NCORES)), trace=_trace
    )
    _results[0] = res
    return assemble_output(res.results)
